# revision 1
# baseline (speedup 1.0000x reference)
"""Trainium2 Bass kernel for Llama4TextExperts (MoE expert MLP chain).

Problem: E=8 experts, T=2048 tokens/expert, H=2048 hidden, D=4096 intermediate.
  hs (E*T, H) -> per expert e: g = hs_e @ Wg_e; u = hs_e @ Wu_e;
  f = u * silu(g); y_e = f @ Wd_e  -> out (E*T, H), all fp32.

Sharding: expert-parallel, 1 expert per NeuronCore (8 cores).

Per-core kernel design:
  - Host pre-transposes hs_e -> xT [H, T] so the stage-1 moving operand has
    the contraction dim (H) on partitions. All matmuls run as float32r
    (full PE rate, 1 cycle/row, at moving free-dim >= 256).
  - Loop over T in tiles of TT=512 tokens:
      stage 1: for each of 32 d-tiles (128 wide): psum_g/psum_u accumulate
        16 matmuls over h-chunks (lhsT = W[h,d] 128x128 stationary,
        rhs = xT[h, t] 128x512 moving). silu on ScalarE, f = silu(g)*u on
        VectorE -> f_T[d] SBUF tiles [128(d) x 512(t)].
      stage 2: for each of 4 h-chunks (512 wide): for each of 4 t-subtiles
        (128): psum_y accumulates 32 matmuls over d (lhsT = f_T[d][:, ts]
        128x128, rhs = Wd[d, h] 128x512 moving) -> copy -> DMA out.
  - Weights are streamed from HBM once per t-tile (96MB/t-tile); DMA
    overlaps PE via double-buffered pools.
"""

import os
import sys

for _p in ("/opt/trn_rl_repo",):
    if _p not in sys.path and os.path.isdir(_p):
        sys.path.insert(0, _p)

import numpy as np
from ml_dtypes import bfloat16 as bf16

E = 8
T = 2048
H = 2048
D = 4096

_CACHE = {}


def _build_bass(H_=H, D_=D, T_=T, TT=512):
    """Build the single-core Bass module (same program for all 8 cores)."""
    import concourse.bass as bass
    import concourse.mybir as mybir
    from concourse.tile import TileContext

    f32 = mybir.dt.float32
    f32r = mybir.dt.float32r
    bf16 = mybir.dt.bfloat16
    P = 128
    N_H = H_ // P            # h-chunks (16)
    N_D = D_ // P            # d-tiles (32)
    N_TT = T_ // TT          # t-tiles (4)
    TS = TT // P             # t-subtiles per t-tile (4)
    HC = 512                 # stage-2 output h-chunk width
    N_HC = H_ // HC          # 4
    WGD = 256                # wg/wu d-width per load (2 d-tiles)
    WD_DCH = 8               # wd d-chunks per load tile

    nc = bass.Bass(trn_type="TRN2")

    xT = nc.declare_dram_parameter("xT", [H_, T_], bf16, isOutput=False)
    wg = nc.declare_dram_parameter("wg", [H_, D_], bf16, isOutput=False)
    wu = nc.declare_dram_parameter("wu", [H_, D_], bf16, isOutput=False)
    wd = nc.declare_dram_parameter("wd", [D_, H_], f32r, isOutput=False)
    y = nc.declare_dram_parameter("y", [T_, H_], f32, isOutput=True)

    xT_r = xT[:].rearrange("(n p) t -> p n t", p=P)    # [128, N_H, T]
    wg_r = wg[:].rearrange("(n p) d -> p n d", p=P)    # [128, N_H, D]
    wu_r = wu[:].rearrange("(n p) d -> p n d", p=P)
    wd_r = wd[:].rearrange("(n p) h -> p n h", p=P)    # [128, N_D, H]
    y_r = y[:].rearrange("(n p) h -> p n h", p=P)      # [128, T//128, H]

    with TileContext(nc) as tc:
        with (
            tc.tile_pool(name="xpool", bufs=1) as xpool,
            tc.tile_pool(name="wpool", bufs=2) as wpool,
            tc.tile_pool(name="wdpool", bufs=3) as wdpool,
            tc.tile_pool(name="fpool", bufs=N_D) as fpool,
            tc.tile_pool(name="spool", bufs=3) as spool,
            tc.tile_pool(name="ypool", bufs=4) as ypool,
            tc.tile_pool(name="pgu", bufs=2, space="PSUM") as pgu,
            tc.tile_pool(name="py", bufs=4, space="PSUM") as py,
        ):
            for tt in range(N_TT):
                tsl = slice(tt * TT, (tt + 1) * TT)
                # ---- load xT[:, t-tile] : [128, N_H, TT]
                x_t = xpool.tile([P, N_H, TT], bf16, tag="x")
                nc.sync.dma_start(out=x_t, in_=xT_r[:, :, tsl])

                # ---- stage 1: gate/up + swiglu, d-tile at a time
                f_tiles = []
                for dt in range(N_D):
                    dw = dt % (WGD // P)   # position inside current weight load
                    if dw == 0:
                        dsl = slice(dt * P, dt * P + WGD)
                        wg_t = wpool.tile([P, N_H, WGD], bf16, tag="wg")
                        wu_t = wpool.tile([P, N_H, WGD], bf16, tag="wu")
                        nc.sync.dma_start(out=wg_t, in_=wg_r[:, :, dsl])
                        nc.sync.dma_start(out=wu_t, in_=wu_r[:, :, dsl])
                    psum_g = pgu.tile([P, TT], f32, tag="pg")
                    psum_u = pgu.tile([P, TT], f32, tag="pu")
                    for h in range(N_H):
                        nc.tensor.matmul(
                            psum_g,
                            lhsT=wg_t[:, h, dw * P:(dw + 1) * P],
                            rhs=x_t[:, h, :],
                            start=(h == 0), stop=(h == N_H - 1),
                        )
                    for h in range(N_H):
                        nc.tensor.matmul(
                            psum_u,
                            lhsT=wu_t[:, h, dw * P:(dw + 1) * P],
                            rhs=x_t[:, h, :],
                            start=(h == 0), stop=(h == N_H - 1),
                        )
                    s_t = spool.tile([P, TT], f32, tag="s")
                    nc.scalar.activation(
                        out=s_t, in_=psum_g,
                        func=mybir.ActivationFunctionType.Silu,
                    )
                    f_t = fpool.tile([P, TT], f32r, tag="f")
                    nc.vector.tensor_mul(f_t, s_t, psum_u)
                    f_tiles.append(f_t)

                # ---- stage 2: y[t, h] = sum_d f_T[d, t] * wd[d, h]
                # ts loop inside the d-group loop: one wd tile alive at a
                # time, TS psum banks accumulate partials across d-groups.
                n_dg = N_D // WD_DCH
                for hc in range(N_HC):
                    psum_ys = [py.tile([P, HC], f32, name=f"py{i}", tag="py")
                               for i in range(TS)]
                    for dg in range(n_dg):
                        wd_t = wdpool.tile([P, WD_DCH, HC], f32r, tag="wd")
                        nc.sync.dma_start(
                            out=wd_t,
                            in_=wd_r[:, dg * WD_DCH:(dg + 1) * WD_DCH,
                                     hc * HC:(hc + 1) * HC],
                        )
                        for ts in range(TS):
                            for dc in range(WD_DCH):
                                dt = dg * WD_DCH + dc
                                nc.tensor.matmul(
                                    psum_ys[ts],
                                    lhsT=f_tiles[dt][:, ts * P:(ts + 1) * P],
                                    rhs=wd_t[:, dc, :],
                                    start=(dt == 0), stop=(dt == N_D - 1),
                                )
                    for ts in range(TS):
                        y_sb = ypool.tile([P, HC], f32, tag="y")
                        nc.scalar.copy(out=y_sb, in_=psum_ys[ts])
                        nc.sync.dma_start(
                            out=y_r[:, tt * TS + ts, hc * HC:(hc + 1) * HC],
                            in_=y_sb,
                        )
    _split_matmul_waits(nc)
    return nc


def _split_matmul_waits(nc):
    """walrus splits fp32r Matmult into LDW+MM and moves the Matmult's sync
    waits onto the generated LW struct, which has room for only one wait.
    Hoist every Matmult's waits onto a PE InstNoOp inserted just before it."""
    import concourse.mybir as mybir

    for f in nc.m.functions:
        for bb in f.blocks:
            insts = list(bb.instructions)
            out = []
            n_nops = 0
            for ins in insts:
                si = ins.sync_info
                tname = type(ins).__name__
                if (
                    si is not None
                    and len(si.on_wait) > (1 if tname != "InstMatmult" else 0)
                ):
                    keep = [] if tname == "InstMatmult" else [si.on_wait[-1]]
                    hoist = si.on_wait if tname == "InstMatmult" else si.on_wait[:-1]
                    for i, w in enumerate(hoist):
                        nop = mybir.InstNoOp(
                            name=f"{ins.name}-waitnop{i}",
                            engine=ins.engine,
                            ins=[],
                            outs=[],
                            sync_info=mybir.SyncInfo(
                                on_wait=[w], on_update=[]
                            ),
                        )
                        out.append(nop)
                        n_nops += 1
                    ins.sync_info = mybir.SyncInfo(
                        on_wait=keep, on_update=list(si.on_update)
                    )
                out.append(ins)
            if n_nops:
                bb.instructions = out


def make_in_maps(hidden_states, gate_proj, up_proj, down_proj):
    hs = np.ascontiguousarray(hidden_states, dtype=np.float32).reshape(E, T, H)
    in_maps = []
    for e in range(E):
        in_maps.append({
            "xT": np.ascontiguousarray(hs[e].T).astype(bf16),
            "wg": np.ascontiguousarray(gate_proj[e], dtype=np.float32).astype(bf16),
            "wu": np.ascontiguousarray(up_proj[e], dtype=np.float32).astype(bf16),
            "wd": np.ascontiguousarray(down_proj[e], dtype=np.float32),
        })
    return in_maps


def kernel(hidden_states, gate_proj, up_proj, down_proj):
    from concourse.bass_utils import run_bass_kernel_spmd

    in_maps = make_in_maps(hidden_states, gate_proj, up_proj, down_proj)
    if "nc" not in _CACHE:
        _CACHE["nc"] = _build_bass()
    nc = _CACHE["nc"]

    res = run_bass_kernel_spmd(nc, in_maps, core_ids=list(range(E)))
    out = np.concatenate([res.results[e]["y"] for e in range(E)], axis=0)
    return out.astype(np.float32)


if __name__ == "__main__":
    # smoke: build only
    nc = _build_bass()
    print("built ok, instructions:", len(nc.inst_map))



# revision 3
# speedup vs baseline: 1.0639x; 1.0639x over previous
"""Trainium2 Bass kernel for Llama4TextExperts (MoE expert MLP chain).

Problem: E=8 experts, T=2048 tokens/expert, H=2048 hidden, D=4096 intermediate.
  hs (E*T, H) -> per expert e: g = hs_e @ Wg_e; u = hs_e @ Wu_e;
  f = u * silu(g); y_e = f @ Wd_e  -> out (E*T, H), all fp32.

Sharding: expert-parallel, 1 expert per NeuronCore (8 cores).

Per-core kernel design (v2):
  - Host pre-transposes hs_e -> xT [H, T] so the stage-1 moving operand has
    the contraction dim (H) on partitions. All matmuls are bf16 (full PE
    rate, FWL active on every LDWEIGHTS since no operand is fp32).
  - Loop over T in tiles of TT=512 tokens:
      stage 1: for each of 32 d-tiles (128 wide): psum_g/psum_u accumulate
        16 matmuls over h-chunks (lhsT = W[h,d] 128x128 stationary,
        rhs = xT[h, t] 128x512 moving). silu on ScalarE, f = silu(g)*u on
        VectorE -> f_T[d] SBUF tiles [128(d) x 512(t)] in bf16.
      stage 2 (ts-outer): for each of 4 h-chunks (512 wide): preload the 4
        wd d-group tiles, then for each of 4 t-subtiles (128): psum_y
        accumulates 32 matmuls over d -> copy -> DMA out. ts-outer drains
        each psum bank early so the tail is short.
  - x tiles are split per h-chunk and issued on the qAct HWDGE ring
    (nc.scalar.dma_start) together with the y output stores, while the
    weight streams ride qSP (nc.sync) - two independent DMA queues.
  - The first weight group of t-tile 0 is 128 wide and split per h-chunk so
    the first matmul can issue after ~160KB of DMA instead of ~4MB.
  - x for t-tile tt+1 is prefetched at the top of stage 2 of tt, ahead of
    the wd loads in queue order.
"""

import os
import sys

for _p in ("/opt/trn_rl_repo",):
    if _p not in sys.path and os.path.isdir(_p):
        sys.path.insert(0, _p)

import numpy as np
from ml_dtypes import bfloat16 as bf16

E = 8
T = 2048
H = 2048
D = 4096

_CACHE = {}


def _build_bass(H_=H, D_=D, T_=T, TT=512):
    """Build the single-core Bass module (same program for all 8 cores)."""
    import concourse.bass as bass
    import concourse.mybir as mybir
    from concourse.tile import TileContext

    f32 = mybir.dt.float32
    bf = mybir.dt.bfloat16
    P = 128
    N_H = H_ // P            # h-chunks (16)
    N_D = D_ // P            # d-tiles (32)
    N_TT = T_ // TT          # t-tiles (4)
    TS = TT // P             # t-subtiles per t-tile (4)
    HC = 512                 # stage-2 output h-chunk width
    N_HC = H_ // HC          # 4
    WGD = 256                # wg/wu d-width per steady-state load
    WD_DCH = 8               # wd d-chunks per load tile
    N_DG = N_D // WD_DCH     # 4

    nc = bass.Bass(trn_type="TRN2")

    xT = nc.declare_dram_parameter("xT", [H_, T_], bf, isOutput=False)
    wg = nc.declare_dram_parameter("wg", [H_, D_], bf, isOutput=False)
    wu = nc.declare_dram_parameter("wu", [H_, D_], bf, isOutput=False)
    wd = nc.declare_dram_parameter("wd", [D_, H_], bf, isOutput=False)
    y = nc.declare_dram_parameter("y", [T_, H_], f32, isOutput=True)

    xT_r = xT[:].rearrange("(n p) t -> p n t", p=P)    # [128, N_H, T]
    wg_r = wg[:].rearrange("(n p) d -> p n d", p=P)    # [128, N_H, D]
    wu_r = wu[:].rearrange("(n p) d -> p n d", p=P)
    wd_r = wd[:].rearrange("(n p) h -> p n h", p=P)    # [128, N_D, H]
    y_r = y[:].rearrange("(n p) h -> p n h", p=P)      # [128, T//128, H]

    # weight-group schedule per t-tile: (d_start, width, split_per_h)
    def wgroups(tt):
        if tt == 0:
            gs = [(0, P, True)]
            d0 = P
            while d0 < D_:
                w = min(WGD, D_ - d0)
                gs.append((d0, w, False))
                d0 += w
            return gs
        return [(k * WGD, WGD, False) for k in range(D_ // WGD)]

    with TileContext(nc) as tc:
        with (
            tc.tile_pool(name="xpool", bufs=2) as xpool,
            tc.tile_pool(name="wpool", bufs=2) as wpool,
            tc.tile_pool(name="wspool", bufs=1) as wspool,
            tc.tile_pool(name="wdpool", bufs=2) as wdpool,
            tc.tile_pool(name="fpool", bufs=N_D) as fpool,
            tc.tile_pool(name="spool", bufs=3) as spool,
            tc.tile_pool(name="ypool", bufs=4) as ypool,
            tc.tile_pool(name="pgu", bufs=2, space="PSUM") as pgu,
            tc.tile_pool(name="py", bufs=4, space="PSUM") as py,
        ):
            def load_x(tt):
                tsl = slice(tt * TT, (tt + 1) * TT)
                xs = []
                for h in range(N_H):
                    x_h = xpool.tile([P, TT], bf, tag=f"x{h}")
                    nc.scalar.dma_start(out=x_h, in_=xT_r[:, h, tsl])
                    xs.append(x_h)
                return xs

            x_cur = load_x(0)
            for tt in range(N_TT):
                # ---- stage 1: gate/up + swiglu, d-tile at a time
                f_tiles = []
                for (d0, w, split) in wgroups(tt):
                    dsl = slice(d0, d0 + w)
                    if split:
                        wg_hs = [wspool.tile([P, w], bf, name=f"wg0h{h}",
                                             tag=f"wg0h{h}")
                                 for h in range(N_H)]
                        wu_hs = [wspool.tile([P, w], bf, name=f"wu0h{h}",
                                             tag=f"wu0h{h}")
                                 for h in range(N_H)]
                        for h in range(N_H):
                            nc.sync.dma_start(out=wg_hs[h],
                                              in_=wg_r[:, h, dsl])
                        for h in range(N_H):
                            nc.sync.dma_start(out=wu_hs[h],
                                              in_=wu_r[:, h, dsl])
                    else:
                        tag = "wg" if w == WGD else "wge"
                        wg_t = wpool.tile([P, N_H, w], bf, tag=tag)
                        wu_t = wpool.tile([P, N_H, w], bf, tag="u" + tag)
                        nc.sync.dma_start(out=wg_t, in_=wg_r[:, :, dsl])
                        nc.sync.dma_start(out=wu_t, in_=wu_r[:, :, dsl])
                    for dw in range(w // P):
                        dt = d0 // P + dw
                        psum_g = pgu.tile([P, TT], f32, tag="pg")
                        psum_u = pgu.tile([P, TT], f32, tag="pu")
                        for h in range(N_H):
                            lhs = (wg_hs[h] if split
                                   else wg_t[:, h, dw * P:(dw + 1) * P])
                            nc.tensor.matmul(
                                psum_g, lhsT=lhs, rhs=x_cur[h],
                                start=(h == 0), stop=(h == N_H - 1),
                            )
                        for h in range(N_H):
                            lhs = (wu_hs[h] if split
                                   else wu_t[:, h, dw * P:(dw + 1) * P])
                            nc.tensor.matmul(
                                psum_u, lhsT=lhs, rhs=x_cur[h],
                                start=(h == 0), stop=(h == N_H - 1),
                            )
                        s_t = spool.tile([P, TT], f32, tag="s")
                        nc.scalar.activation(
                            out=s_t, in_=psum_g,
                            func=mybir.ActivationFunctionType.Silu,
                        )
                        f_t = fpool.tile([P, TT], bf, tag="f")
                        nc.vector.tensor_mul(f_t, s_t, psum_u)
                        f_tiles.append(f_t)

                # ---- prefetch next t-tile's x ahead of the wd loads
                if tt + 1 < N_TT:
                    x_next = load_x(tt + 1)
                else:
                    x_next = None

                # ---- stage 2: y[t, h] = sum_d f_T[d, t] * wd[d, h]
                # ts-outer: each psum bank finishes all 32 d accumulations,
                # then drains (copy + DMA) while the next ts is computing.
                for hc in range(N_HC):
                    hsl = slice(hc * HC, (hc + 1) * HC)
                    wd_ts = []
                    for dg in range(N_DG):
                        wd_t = wdpool.tile([P, WD_DCH, HC], bf, tag=f"wd{dg}")
                        nc.sync.dma_start(
                            out=wd_t,
                            in_=wd_r[:, dg * WD_DCH:(dg + 1) * WD_DCH, hsl],
                        )
                        wd_ts.append(wd_t)
                    for ts in range(TS):
                        psum_y = py.tile([P, HC], f32, tag="py")
                        for dg in range(N_DG):
                            for dc in range(WD_DCH):
                                dt = dg * WD_DCH + dc
                                nc.tensor.matmul(
                                    psum_y,
                                    lhsT=f_tiles[dt][:, ts * P:(ts + 1) * P],
                                    rhs=wd_ts[dg][:, dc, :],
                                    start=(dt == 0), stop=(dt == N_D - 1),
                                )
                        y_sb = ypool.tile([P, HC], f32, tag="y")
                        nc.scalar.copy(out=y_sb, in_=psum_y)
                        nc.scalar.dma_start(
                            out=y_r[:, tt * TS + ts, hsl], in_=y_sb,
                        )
                x_cur = x_next
    _split_matmul_waits(nc)
    return nc


def _split_matmul_waits(nc):
    """walrus splits Matmult into LDW+MM and moves the Matmult's sync
    waits onto the generated LW struct, which has room for only one wait.
    Hoist every Matmult's waits onto a PE InstNoOp inserted just before it."""
    import concourse.mybir as mybir

    for f in nc.m.functions:
        for bb in f.blocks:
            insts = list(bb.instructions)
            out = []
            n_nops = 0
            for ins in insts:
                si = ins.sync_info
                tname = type(ins).__name__
                if (
                    si is not None
                    and len(si.on_wait) > (1 if tname != "InstMatmult" else 0)
                ):
                    keep = [] if tname == "InstMatmult" else [si.on_wait[-1]]
                    hoist = si.on_wait if tname == "InstMatmult" else si.on_wait[:-1]
                    for i, w in enumerate(hoist):
                        nop = mybir.InstNoOp(
                            name=f"{ins.name}-waitnop{i}",
                            engine=ins.engine,
                            ins=[],
                            outs=[],
                            sync_info=mybir.SyncInfo(
                                on_wait=[w], on_update=[]
                            ),
                        )
                        out.append(nop)
                        n_nops += 1
                    ins.sync_info = mybir.SyncInfo(
                        on_wait=keep, on_update=list(si.on_update)
                    )
                out.append(ins)
            if n_nops:
                bb.instructions = out


def make_in_maps(hidden_states, gate_proj, up_proj, down_proj):
    hs = np.ascontiguousarray(hidden_states, dtype=np.float32).reshape(E, T, H)
    in_maps = []
    for e in range(E):
        in_maps.append({
            "xT": np.ascontiguousarray(hs[e].T).astype(bf16),
            "wg": np.ascontiguousarray(gate_proj[e], dtype=np.float32).astype(bf16),
            "wu": np.ascontiguousarray(up_proj[e], dtype=np.float32).astype(bf16),
            "wd": np.ascontiguousarray(down_proj[e], dtype=np.float32).astype(bf16),
        })
    return in_maps


def kernel(hidden_states, gate_proj, up_proj, down_proj):
    from concourse.bass_utils import run_bass_kernel_spmd

    in_maps = make_in_maps(hidden_states, gate_proj, up_proj, down_proj)
    if "nc" not in _CACHE:
        _CACHE["nc"] = _build_bass()
    nc = _CACHE["nc"]

    res = run_bass_kernel_spmd(nc, in_maps, core_ids=list(range(E)))
    out = np.concatenate([res.results[e]["y"] for e in range(E)], axis=0)
    return out.astype(np.float32)


if __name__ == "__main__":
    # smoke: build only
    nc = _build_bass()
    print("built ok, instructions:", len(nc.inst_map))
